# revision 71
# baseline (speedup 1.0000x reference)
"""Single-head causal attention (S=2048, B=8, D=1024) for 8 TRN2 NeuronCores.

Sharding: data-parallel over the batch dim - core c computes batch element c.

fp8 fast path (causal variant):
  - All heavy matmuls run as fp8(e4m3) DoubleRow: each instruction contracts
    256 rows (2x128 pair-interleaved) at 0.5 cycles per output column.
  - Host pre-quantizes q/k/v and the folded weights B = SCALE*Wk^T@Wq (scaled
    by SB_SCALE) and C = Wv^T@Wo^T (scaled by SC_SCALE) to fp8.
  - Accuracy: causal softmax rows with few keys amplify quantization noise,
    so a "clean" prefix covers the start of the sequence: K/V projections
    use 3-term hi/lo fp8 splits (hi*hi + lo*hi + hi*lo, ~fp16 accuracy at
    0.75x fp16's cost) for keys j < JCL (=128), and the attention for rows
    i < ICL (=128) runs in fp16; everything else is single-term fp8. exp()
    uses a global -CSHIFT shift (the denominator sums the same quantized
    weights, so the shift and all common-mode quantization cancel).
  - Softmax denominator rides the PV DoubleRow accumulation as an extra
    8-column all-(SC_SCALE) rhs; 1/l is applied in the epilogue (DVE muls,
    ACT only where exp is idle), output stored fp16.
  - G (projected keys) and v' (output-projected values) stay SBUF-resident;
    no DRAM scratch roundtrip. Dummy warmup matmuls bridge the first-DMA
    latency so the PE p-state ramp completes before real work.

Legacy fp32r path kept for non-causal masks.
"""

import math
from contextlib import ExitStack

import numpy as np
import ml_dtypes

import concourse.bass as bass
import concourse.mybir as mybir
import concourse.tile as tile
from concourse import bacc
from concourse.bass_utils import run_bass_kernel_spmd
from concourse.masks import make_identity

S, B, D = 2048, 8, 1024
P = 128
DI = D // P  # 8 contraction chunks
DI2 = D // 256  # 4 DoubleRow contraction chunks
JC = S // P  # 16 key chunks
NSB = 4  # query superblocks
SBW = S // NSB  # 512 queries per superblock
SCALE = 1.0 / math.sqrt(D)
CORES = list(range(8))
F32 = mybir.dt.float32
F32R = mybir.dt.float32r
F16 = mybir.dt.float16
F8 = mybir.dt.float8e4
NPF8 = ml_dtypes.float8_e4m3
DR = mybir.MatmulPerfMode.DoubleRow

SB_SCALE = 512.0  # B-weight scale (entries ~1e-3 would be subnormal in fp8)
SC_SCALE = 16.0  # C-weight scale
CSHIFT = 2.0  # global score shift before exp (keeps p in fp8 range)
CLEAN = 256  # attention rows < CLEAN run in fp16
CJC = CLEAN // P  # 2 attention-clean j-chunks
NF8 = S - CLEAN  # fp8 columns of q
JCL = 128  # projections computed cleanly (3-term) only for j < JCL
NF8K = S - JCL  # fp8 columns of k/v


_cache: dict[str, object] = {}


def _build_causal_fp8():
    nc = bacc.Bacc("TRN2", num_devices=len(CORES))

    qin8 = nc.dram_tensor("qin8", [P, DI2, 2, NF8], F8, kind="ExternalInput").ap()
    qin16 = nc.dram_tensor("qin16", [P, DI, CLEAN], F16, kind="ExternalInput").ap()
    kin8 = nc.dram_tensor("kin8", [P, DI2, 2, NF8K], F8, kind="ExternalInput").ap()
    kc8h = nc.dram_tensor("kc8h", [P, DI2, 2, JCL], F8, kind="ExternalInput").ap()
    kc8l = nc.dram_tensor("kc8l", [P, DI2, 2, JCL], F8, kind="ExternalInput").ap()
    vin8 = nc.dram_tensor("vin8", [P, DI2, 2, NF8K], F8, kind="ExternalInput").ap()
    vc8h = nc.dram_tensor("vc8h", [P, DI2, 2, JCL], F8, kind="ExternalInput").ap()
    vc8l = nc.dram_tensor("vc8l", [P, DI2, 2, JCL], F8, kind="ExternalInput").ap()
    wb8 = nc.dram_tensor("wb8", [P, DI2, 2, D], F8, kind="ExternalInput").ap()
    wb8l = nc.dram_tensor("wb8l", [P, DI2, 2, D], F8, kind="ExternalInput").ap()
    wc8 = nc.dram_tensor("wc8", [P, DI2, 2, D], F8, kind="ExternalInput").ap()
    wc8l = nc.dram_tensor("wc8l", [P, DI2, 2, D], F8, kind="ExternalInput").ap()
    wvec = nc.dram_tensor("wvec", [P, JC], F32, kind="ExternalInput").ap()
    out = nc.dram_tensor("out", [S, D], F16, kind="ExternalOutput").ap()

    with tile.TileContext(nc) as tc, ExitStack() as ctx:
        pool_const = ctx.enter_context(tc.tile_pool(name="const", bufs=1))
        pool_g = ctx.enter_context(tc.tile_pool(name="gres", bufs=1))
        pool_v = ctx.enter_context(tc.tile_pool(name="vres", bufs=1))
        pool_q = ctx.enter_context(tc.tile_pool(name="qres", bufs=1))
        pool_pt0 = ctx.enter_context(tc.tile_pool(name="pt0", bufs=1))
        pool_y = ctx.enter_context(tc.tile_pool(name="yp", bufs=4))
        pool_small = ctx.enter_context(tc.tile_pool(name="smal", bufs=4))

        wv_t = pool_const.tile([P, JC], F32)
        ones8_t = pool_const.tile([P, 2, 8], F8)
        ones16_t = pool_const.tile([P, 8], F16)
        garb_t = pool_const.tile([P, P], F16)  # never written: PE warmup fuel

        g8 = pool_g.tile([P, DI, S], F8)
        g16 = pool_g.tile([P, DI, JCL], F16)
        v8 = pool_v.tile([P, JC, D], F8)
        v16 = pool_v.tile([P, 1, D], F16)
        q8 = pool_q.tile([P, DI2, 2, NF8], F8)
        q16 = pool_q.tile([P, DI, CLEAN], F16)

        # cast-engine alternation
        _ce = [0]

        def cast(dst, src, force=None):
            e = force if force is not None else ("dve" if _ce[0] % 2 == 0 else "act")
            if e == "dve":
                nc.vector.tensor_copy(dst, src)
            else:
                nc.scalar.copy(dst, src)
            _ce[0] += 1


        def emit_epilogue(sb, icl, l_ps, yps, last=False):
            rinv = pool_small.tile([P, 1], F32, tag="ri", name=f"ri{sb}_{icl}")
            nc.vector.reciprocal(rinv[:], l_ps[:, 0:1])
            r0 = sb * SBW + icl * P
            if last:
                # final tile: both halves in parallel on separate engines
                # and queues to shorten the kernel tail
                ysb = pool_y.tile([P, D], F16, tag="y", name=f"y{sb}_{icl}")
                nc.vector.tensor_scalar_mul(ysb[:, :SBW], yps[0][:], rinv[:, 0:1])
                nc.scalar.mul(ysb[:, SBW:], yps[1][:], rinv[:, 0:1])
                nc.sync.dma_start(out[r0 : r0 + P, :SBW], ysb[:, :SBW])
                nc.scalar.dma_start(out[r0 : r0 + P, SBW:], ysb[:, SBW:])
            else:
                ysb = pool_y.tile([P, D], F16, tag="y", name=f"y{sb}_{icl}")
                nc.vector.tensor_scalar_mul(ysb[:, :SBW], yps[0][:], rinv[:, 0:1])
                if sb == NSB - 1:
                    # ACT is exp-free during sb3 PV; share the mul load
                    nc.scalar.mul(ysb[:, SBW:], yps[1][:], rinv[:, 0:1])
                else:
                    # keep ACT free for exp: both muls on DVE
                    nc.vector.tensor_scalar_mul(
                        ysb[:, SBW:], yps[1][:], rinv[:, 0:1]
                    )
                nc.sync.dma_start(out[r0 : r0 + P, :], ysb[:])

        # ---------------- phase A: projections ----------------
        with (
            tc.tile_pool(name="wts", bufs=1) as pool_w,
            tc.tile_pool(name="ins", bufs=1) as pool_in,
            tc.tile_pool(name="pps", bufs=8, space="PSUM") as psum_a,
        ):
            kin8_t = pool_in.tile([P, DI2, 2, NF8K], F8, name="kin8")
            kc8h_t = pool_in.tile([P, DI2, 2, JCL], F8, name="kc8h")
            kc8l_t = pool_in.tile([P, DI2, 2, JCL], F8, name="kc8l")
            vin8_t = pool_in.tile([P, DI2, 2, NF8K], F8, name="vin8")
            vc8h_t = pool_in.tile([P, DI2, 2, JCL], F8, name="vc8h")
            vc8l_t = pool_in.tile([P, DI2, 2, JCL], F8, name="vc8l")
            wb8_t = pool_w.tile([P, DI2, 2, D], F8, name="wb8")
            wb8l_t = pool_w.tile([P, DI2, 2, D], F8, name="wb8l")
            wc8_t = pool_w.tile([P, DI2, 2, D], F8, name="wc8")
            wc8l_t = pool_w.tile([P, DI2, 2, D], F8, name="wc8l")

            # scalar queue: wb8 per-md chunks (K-clean consumes md-outer),
            # then wc8; ACT is cast-free until ~6us
            nc.scalar.dma_start(wb8_t[:, 0, :, :SBW], wb8[:, 0, :, :SBW])
            nc.scalar.dma_start(wb8_t[:, 0, :, SBW:], wb8[:, 0, :, SBW:])
            for md in range(1, DI2):
                nc.scalar.dma_start(wb8_t[:, md], wb8[:, md])
            # sync queue: small clean-prefix hi/lo inputs + wc8 + q16 + wvec
            nc.sync.dma_start(kc8h_t[:], kc8h[:])
            nc.sync.dma_start(kc8l_t[:], kc8l[:])
            nc.sync.dma_start(vc8h_t[:], vc8h[:])
            nc.sync.dma_start(vc8l_t[:], vc8l[:])
            for md in range(DI2):
                nc.sync.dma_start(wc8_t[:, md], wc8[:, md])
            nc.sync.dma_start(q16[:], qin16[:])
            nc.sync.dma_start(wv_t[:], wvec[:])

            # gpsimd queue: lo-weights per-md first (K-clean term 3), then
            # kin8 chunks in the jb-outer consumption order
            kblocks = [(128, 640), (640, 1152), (1152, 1664),
                       (1664, 1920), (0, 128)]
            nc.gpsimd.memset(ones8_t[:], SC_SCALE)
            nc.gpsimd.memset(ones16_t[:], SC_SCALE)
            for md in range(DI2):
                nc.gpsimd.dma_start(wb8l_t[:, md], wb8l[:, md])
            nc.gpsimd.dma_start(kin8_t[:, :, :, 128:640], kin8[:, :, :, 128:640])
            nc.gpsimd.dma_start(wc8l_t[:], wc8l[:])
            for lo, hi in kblocks[1:]:
                nc.gpsimd.dma_start(kin8_t[:, :, :, lo:hi], kin8[:, :, :, lo:hi])
            HN = NF8 // 2  # 896
            nc.gpsimd.dma_start(vin8_t[:, :, :, :HN], vin8[:, :, :, :HN])
            nc.gpsimd.dma_start(vin8_t[:, :, :, HN:], vin8[:, :, :, HN:])
            nc.gpsimd.dma_start(q8[:, :, :, :HN], qin8[:, :, :, :HN])
            nc.gpsimd.dma_start(q8[:, :, :, HN:], qin8[:, :, :, HN:])

            # PE warmup: dummy matmuls on an uninitialized (all-zero) tile
            # bridge the first-DMA latency (~3us) and keep the PE p-state
            # ramp counting so the real work starts at full clock.
            warm_ps = psum_a.tile([P, SBW], F32, tag="ps", name="warm")
            nc.vector.memset(garb_t[:], 0.0)
            for _ in range(4):
                nc.tensor.matmul(
                    warm_ps[:, :64], garb_t[:], garb_t[:, :64], start=True,
                    stop=True,
                )

            # K-proj clean: G^T[m-chunk, j<JCL] via 3-term hi/lo fp8 DR
            # (hi*hi + lo_k*hi + hi*lo_B: ~fp16 accuracy at 0.75x the cost).
            # md-outer so the first matmuls only need the first wb8 chunk.
            for mh in range(2):
                kc_ps = [
                    psum_a.tile([P, SBW], F32, tag="ps", name=f"kc{mh}_{m4}")
                    for m4 in range(4)
                ]
                for md in range(DI2):
                    for m4 in range(4):
                        m = 4 * mh + m4
                        for t, (wt, kt) in enumerate(
                            ((wb8_t, kc8h_t), (wb8_t, kc8l_t), (wb8l_t, kc8h_t))
                        ):
                            nc.tensor.matmul(
                                kc_ps[m4][:, :JCL],
                                wt[:, md, :, m * P : (m + 1) * P],
                                kt[:, md],
                                start=md == 0 and t == 0,
                                stop=md == DI2 - 1 and t == 2,
                                perf_mode=DR,
                            )
                for m4 in range(4):
                    m = 4 * mh + m4
                    cast(g16[:, m, :], kc_ps[m4][:, :JCL])
                    cast(g8[:, m, :JCL], kc_ps[m4][:, :JCL])

            # K-proj fp8: j >= JCL (kin8 col x = j - JCL)
            # jb-outer so each j-block only needs its own kin8 chunk
            for lo, hi in kblocks:
                for m in range(DI):
                    w = hi - lo
                    ps = psum_a.tile([P, SBW], F32, tag="ps", name=f"kf{m}_{lo}")
                    for md in range(DI2):
                        nc.tensor.matmul(
                            ps[:, :w],
                            wb8_t[:, md, :, m * P : (m + 1) * P],
                            kin8_t[:, md, :, lo:hi],
                            start=md == 0,
                            stop=md == DI2 - 1,
                            perf_mode=DR,
                        )
                    cast(g8[:, m, JCL + lo : JCL + hi], ps[:, :w])

            # V-proj clean: v'[j<JCL, :] via 3-term hi/lo fp8 DR
            vc_ps = [
                psum_a.tile([P, SBW], F32, tag="ps", name=f"vc{je}")
                for je in range(2)
            ]
            for md in range(DI2):
                for jcl in range(1):
                    for eh in range(2):
                        for t, (vt, wt) in enumerate(
                            ((vc8h_t, wc8_t), (vc8l_t, wc8_t), (vc8h_t, wc8l_t))
                        ):
                            nc.tensor.matmul(
                                vc_ps[2 * jcl + eh][:],
                                vt[:, md, :, jcl * P : (jcl + 1) * P],
                                wt[:, md, :, eh * SBW : (eh + 1) * SBW],
                                start=md == 0 and t == 0,
                                stop=md == DI2 - 1 and t == 2,
                                perf_mode=DR,
                            )
            for jcl in range(1):
                for eh in range(2):
                    ps = vc_ps[2 * jcl + eh]
                    cast(v16[:, jcl, eh * SBW : (eh + 1) * SBW], ps[:])
                    cast(v8[:, jcl, eh * SBW : (eh + 1) * SBW], ps[:])

            # V-proj fp8: j >= JCL (vin8 col x = j - JCL)
            for jc in range(1, JC):
                for eh in range(2):
                    ps = psum_a.tile([P, SBW], F32, tag="ps", name=f"vf{jc}_{eh}")
                    for md in range(DI2):
                        nc.tensor.matmul(
                            ps[:],
                            vin8_t[:, md, :, jc * P - JCL : (jc + 1) * P - JCL],
                            wc8_t[:, md, :, eh * SBW : (eh + 1) * SBW],
                            start=md == 0,
                            stop=md == DI2 - 1,
                            perf_mode=DR,
                        )
                    cast(v8[:, jc, eh * SBW : (eh + 1) * SBW], ps[:])

        # ---------------- phase B: attention ----------------
        with (
            tc.tile_pool(name="ptp", bufs=2) as pool_pt,
            tc.tile_pool(name="yps", bufs=4, space="PSUM") as psum_y,
            tc.tile_pool(name="lps", bufs=1, space="PSUM") as psum_l,
            tc.tile_pool(name="qkps", bufs=3, space="PSUM") as psum_qk,
        ):

            # ---- sb0 clean part: rows [0, ICL) in fp16
            pt16 = pool_pt0.tile([P, CJC, CLEAN], F16, name="pt16")
            for jc in range(CJC):
                off = jc * P
                ps = psum_qk.tile([P, SBW], F32, tag="qk", name=f"qkc{jc}")
                for di in range(DI):
                    lh = (g16[:, di, :] if jc == 0
                          else g8[:, di, jc * P : (jc + 1) * P])
                    nc.tensor.matmul(
                        ps[:, off:CLEAN],
                        lh,
                        q16[:, di, off:],
                        start=di == 0,
                        stop=di == DI - 1,
                    )
                nc.scalar.activation(
                    pt16[:, jc, off:],
                    ps[:, off:CLEAN],
                    mybir.ActivationFunctionType.Exp,
                    bias=wv_t[:, jc : jc + 1],
                    scale=1.0 / SB_SCALE,
                )
                nc.gpsimd.affine_select(
                    out=pt16[:, jc, off : off + P],
                    in_=pt16[:, jc, off : off + P],
                    compare_op=mybir.AluOpType.is_ge,
                    fill=0.0,
                    base=0,
                    pattern=[[1, P]],
                    channel_multiplier=-1,
                )
            for icl in range(CJC):
                l_ps = psum_l.tile([P, SBW], F32, tag="l", name=f"l0_{icl}")
                yps = [
                    psum_y.tile([P, SBW], F32, tag="yp", name=f"yp0_{icl}_{eh}")
                    for eh in range(2)
                ]
                for jc in range(icl + 1):
                    lhsT = pt16[:, jc, icl * P : (icl + 1) * P]
                    for eh in range(2):
                        rh = (v16[:, 0, eh * SBW : (eh + 1) * SBW] if jc == 0
                              else v8[:, jc, eh * SBW : (eh + 1) * SBW])
                        nc.tensor.matmul(
                            yps[eh][:],
                            lhsT,
                            rh,
                            start=jc == 0,
                            stop=jc == icl,
                        )
                    nc.tensor.matmul(
                        l_ps[:, :1],
                        lhsT,
                        ones16_t[:, 0:1],
                        start=jc == 0,
                        stop=jc == icl,
                    )
                emit_epilogue(0, icl, l_ps, yps)

            # ---- sb0 fp8 part: rows [ICL, SBW) (q8 col x = i - ICL)
            pt8a = pool_pt0.tile([P, 4, 256], F8, name="pt8a")
            nc.gpsimd.memset(pt8a[:, 3, 0:P], 0.0)
            for jc in range(4):
                off = max(0, jc * P - CLEAN)  # local offset in [0,256)
                ps = psum_qk.tile([P, SBW], F32, tag="qk", name=f"qka{jc}")
                for md in range(DI2):
                    nc.tensor.matmul(
                        ps[:, off:256],
                        g8[:, 2 * md : 2 * md + 2, jc * P : (jc + 1) * P],
                        q8[:, md, :, off:256],
                        start=md == 0,
                        stop=md == DI2 - 1,
                        perf_mode=DR,
                    )
                nc.scalar.activation(
                    pt8a[:, jc, off:],
                    ps[:, off:256],
                    mybir.ActivationFunctionType.Exp,
                    bias=wv_t[:, jc : jc + 1],
                    scale=1.0 / SB_SCALE,
                )
                if jc * P >= CLEAN:
                    nc.gpsimd.affine_select(
                        out=pt8a[:, jc, off : off + P],
                        in_=pt8a[:, jc, off : off + P],
                        compare_op=mybir.AluOpType.is_ge,
                        fill=0.0,
                        base=CLEAN + off - jc * P,
                        pattern=[[1, P]],
                        channel_multiplier=-1,
                    )
            for icl in (2, 3):
                loc = icl * P - CLEAN
                l_ps = psum_l.tile([P, SBW], F32, tag="l", name=f"l0a{icl}")
                yps = [
                    psum_y.tile([P, SBW], F32, tag="yp", name=f"yp0a{icl}_{eh}")
                    for eh in range(2)
                ]
                npair = icl // 2 + 1
                for m in range(npair):
                    lhsT = pt8a[:, 2 * m : 2 * m + 2, loc : loc + P]
                    for eh in range(2):
                        nc.tensor.matmul(
                            yps[eh][:],
                            lhsT,
                            v8[:, 2 * m : 2 * m + 2, eh * SBW : (eh + 1) * SBW],
                            start=m == 0,
                            stop=m == npair - 1,
                            perf_mode=DR,
                        )
                    nc.tensor.matmul(
                        l_ps[:, :1],
                        lhsT,
                        ones8_t[:, :, 0:1],
                        start=m == 0,
                        stop=m == npair - 1,
                        perf_mode=DR,
                    )
                emit_epilogue(0, icl, l_ps, yps)

            # ---- sb1..3: pure fp8. Two pt buffers reused manually
            # (sb3 reuses sb1's tile) so the diagonal-region memsets for
            # both rounds run once, early, off the Pool critical path.
            pt_bufs = {}
            for sb in range(1, NSB):
                nj = 4 * sb + 4
                if sb - 2 in pt_bufs:
                    pt = pt_bufs[sb - 2]
                else:
                    pt = pool_pt.tile([P, JC, SBW], F8, tag="pt", name=f"pt{sb}")
                    pt_bufs[sb] = pt
                    for s2 in (sb, sb + 2):
                        if s2 < NSB:
                            for t in (1, 2, 3):
                                nc.gpsimd.memset(
                                    pt[:, 4 * s2 + t, 0 : t * P], 0.0
                                )
                for jc in range(nj):
                    off = max(0, (jc - 4 * sb) * P)
                    ps = psum_qk.tile([P, SBW], F32, tag="qk", name=f"qk{sb}_{jc}")
                    for md in range(DI2):
                        nc.tensor.matmul(
                            ps[:, off:],
                            g8[:, 2 * md : 2 * md + 2, jc * P : (jc + 1) * P],
                            q8[:, md, :, sb * SBW + off - CLEAN : (sb + 1) * SBW - CLEAN],
                            start=md == 0,
                            stop=md == DI2 - 1,
                            perf_mode=DR,
                        )
                    nc.scalar.activation(
                        pt[:, jc, off:],
                        ps[:, off:],
                        mybir.ActivationFunctionType.Exp,
                        bias=wv_t[:, jc : jc + 1],
                        scale=1.0 / SB_SCALE,
                    )
                    if jc >= 4 * sb:
                        bend = min(off + P, SBW)
                        nc.gpsimd.affine_select(
                            out=pt[:, jc, off:bend],
                            in_=pt[:, jc, off:bend],
                            compare_op=mybir.AluOpType.is_ge,
                            fill=0.0,
                            base=sb * SBW + off - jc * P,
                            pattern=[[1, bend - off]],
                            channel_multiplier=-1,
                        )
                for icl in range(4):
                    ic = 4 * sb + icl
                    npair = ic // 2 + 1
                    l_ps = psum_l.tile([P, SBW], F32, tag="l", name=f"l{sb}_{icl}")
                    yps = [
                        psum_y.tile([P, SBW], F32, tag="yp", name=f"yp{sb}_{icl}_{eh}")
                        for eh in range(2)
                    ]
                    for m in range(npair):
                        lhsT = pt[:, 2 * m : 2 * m + 2, icl * P : (icl + 1) * P]
                        for eh in range(2):
                            nc.tensor.matmul(
                                yps[eh][:],
                                lhsT,
                                v8[:, 2 * m : 2 * m + 2, eh * SBW : (eh + 1) * SBW],
                                start=m == 0,
                                stop=m == npair - 1,
                                perf_mode=DR,
                            )
                        nc.tensor.matmul(
                            l_ps[:, :1],
                            lhsT,
                            ones8_t[:, :, 0:1],
                            start=m == 0,
                            stop=m == npair - 1,
                            perf_mode=DR,
                        )
                    emit_epilogue(
                        sb, icl, l_ps, yps, last=(sb == NSB - 1 and icl == 3)
                    )

    nc.compile()
    return nc


def _host_inputs_fp8(query, key, value, mask, Wq, bq, Wk, bk, Wv, bv, Wo, bo, c):
    q = np.ascontiguousarray(query[:, c, :])
    k = np.ascontiguousarray(key[:, c, :])
    v = np.ascontiguousarray(value[:, c, :])
    Bm = (
        SCALE * SB_SCALE * (Wk.T.astype(np.float64) @ Wq.astype(np.float64))
    ).astype(np.float32)
    Cm = (
        SC_SCALE * (Wv.T.astype(np.float64) @ Wo.T.astype(np.float64))
    ).astype(np.float32)

    def drl(xT):  # [D, N] -> [P, DI2, 2, N] fp32 pair-interleave layout
        n = xT.shape[1]
        return np.ascontiguousarray(xT.reshape(DI2, 2, P, n).transpose(2, 0, 1, 3))

    def dr8(xT):  # fp8 (hi) in DR layout
        return drl(xT).astype(NPF8)

    def dr8hl(xT):  # (hi, lo) fp8 pair in DR layout
        a = drl(xT)
        hi = a.astype(NPF8)
        lo = (a - hi.astype(np.float32)).astype(NPF8)
        return hi, lo

    def pl16(xT, n):  # [D, N] -> [P, DI, n] fp16
        return np.ascontiguousarray(
            xT[:, :n].reshape(DI, P, n).transpose(1, 0, 2)
        ).astype(np.float16)

    qT, kT, vT = q.T, k.T, v.T
    wvec = (SCALE * (k @ (Wk.T @ bq)) - CSHIFT).astype(np.float32)
    kc8h, kc8l = dr8hl(np.ascontiguousarray(kT[:, :JCL]))
    vc8h, vc8l = dr8hl(np.ascontiguousarray(vT[:, :JCL]))
    wb8, wb8l = dr8hl(Bm)
    wc8, wc8l = dr8hl(Cm)
    return {
        "qin8": dr8(np.ascontiguousarray(qT[:, CLEAN:])),
        "qin16": pl16(qT, CLEAN),
        "kin8": dr8(np.ascontiguousarray(kT[:, JCL:])),
        "kc8h": kc8h,
        "kc8l": kc8l,
        "vin8": dr8(np.ascontiguousarray(vT[:, JCL:])),
        "vc8h": vc8h,
        "vc8l": vc8l,
        "wb8": wb8,
        "wb8l": wb8l,
        "wc8": wc8,
        "wc8l": wc8l,
        "wvec": np.ascontiguousarray(wvec.reshape(JC, P).T),
    }


# ---------------------------------------------------------------------------
# legacy fp32r kernel (fallback for non-causal masks)
# ---------------------------------------------------------------------------


def _build_legacy(variant: str):
    """variant: 'full' (no mask), 'masked' (arbitrary 0/1 mask from DRAM)."""
    assert variant in ("full", "masked")
    nc = bacc.Bacc("TRN2", num_devices=len(CORES))

    qin = nc.dram_tensor("qin", [D, S], F32R, kind="ExternalInput").ap()
    kin = nc.dram_tensor("kin", [D, S], F32R, kind="ExternalInput").ap()
    vin = nc.dram_tensor("vin", [D, S], F32R, kind="ExternalInput").ap()
    wkt = nc.dram_tensor("wkt", [D, D], F32R, kind="ExternalInput").ap()
    wvt = nc.dram_tensor("wvt", [D, D], F32R, kind="ExternalInput").ap()
    wvec = nc.dram_tensor("wvec", [P, JC], F32, kind="ExternalInput").ap()
    borep = nc.dram_tensor("borep", [P, D], F32, kind="ExternalInput").ap()
    onesd = nc.dram_tensor("onesd", [P, P], F32R, kind="ExternalInput").ap()
    if variant == "masked":
        maskt = nc.dram_tensor("maskt", [S, S], F32, kind="ExternalInput").ap()
    out = nc.dram_tensor("out", [S, D], F32, kind="ExternalOutput").ap()

    kT_d = nc.dram_tensor("kT_d", [DI, P, S], F32R).ap()

    nj = JC

    with tile.TileContext(nc) as tc, ExitStack() as ctx:
        pool_const = ctx.enter_context(tc.tile_pool(name="const", bufs=1))
        pool_v = ctx.enter_context(tc.tile_pool(name="vres", bufs=1))
        pool_qt = ctx.enter_context(tc.tile_pool(name="qtp", bufs=2))
        pool_kt = ctx.enter_context(tc.tile_pool(name="ktp", bufs=3))

        ident = pool_const.tile([P, P], F32)
        make_identity(nc, ident[:])
        ones_t = pool_const.tile([P, P], F32R)
        wv_t = pool_const.tile([P, JC], F32)
        borep_t = pool_const.tile([P, D], F32)

        v_sb = pool_v.tile([P, JC, D], F32R)

        qt_tiles = {}
        n_kt0 = 3
        kt0_tiles = [
            pool_kt.tile([P, DI, P], F32R, tag="kt", name=f"kt0_{jc}")
            for jc in range(n_kt0)
        ]

        with (
            tc.tile_pool(name="wts", bufs=3) as pool_w,
            tc.tile_pool(name="ins", bufs=2) as pool_in,
            tc.tile_pool(name="stg", bufs=4) as pool_stage,
            tc.tile_pool(name="pps", bufs=4, space="PSUM") as psum_p,
        ):

            def load_weight_half(w_dram, h, split=False):
                wr = w_dram.rearrange("(di p) o -> p di o", p=P)
                wt = pool_w.tile([P, DI, 512], F32R, tag="wt", name=f"w{h}")
                if split:
                    for m in range(4):
                        nc.sync.dma_start(
                            wt[:, :, m * P : (m + 1) * P],
                            wr[:, :, h * 512 + m * P : h * 512 + (m + 1) * P],
                        )
                else:
                    nc.scalar.dma_start(wt[:], wr[:, :, h * 512 : (h + 1) * 512])
                return wt

            def wslice(halves, di, m):
                return halves[m // 4][:, di, (m % 4) * P : (m % 4 + 1) * P]

            def project_T(w_halves, b_tile, x_dram, dst_dram, split_first_tin=False,
                          after_cols=(), after_first_tin=None):
                xr = x_dram.rearrange("(di p) s -> p di s", p=P)
                for jc4 in range(S // 512):
                    tin = pool_in.tile([P, DI, 512], F32R, tag="tin")
                    if jc4 == 0 and split_first_tin:
                        for di in range(DI):
                            nc.gpsimd.dma_start(tin[:, di, :], xr[:, di, 0:512])
                    else:
                        nc.sync.dma_start(
                            tin[:], xr[:, :, jc4 * 512 : (jc4 + 1) * 512]
                        )
                    if jc4 == 0 and after_first_tin is not None:
                        after_first_tin()
                    for m in range(DI):
                        ps = psum_p.tile([P, 512], F32, tag="ps")
                        for di in range(DI):
                            nc.tensor.matmul(
                                ps[:],
                                wslice(w_halves, di, m),
                                tin[:, di, :],
                                start=di == 0,
                                stop=di == DI - 1,
                            )
                        st = pool_stage.tile([P, 512], F32R, tag="st")
                        if b_tile is None:
                            nc.vector.tensor_copy(st[:], ps[:])
                        else:
                            nc.vector.tensor_scalar_add(
                                st[:], ps[:], b_tile[:, m : m + 1]
                            )
                        nc.scalar.dma_start(
                            dst_dram[m, :, jc4 * 512 : (jc4 + 1) * 512], st[:]
                        )
                    if after_cols and jc4 < len(after_cols) and after_cols[jc4]:
                        after_cols[jc4]()

            def prefetch_kt0(a, b):
                for jc in range(a, min(b, n_kt0)):
                    nc.gpsimd.dma_start(
                        kt0_tiles[jc][:],
                        kT_d[:, :, jc * P : (jc + 1) * P].rearrange(
                            "di p j -> p di j"
                        ),
                    )

            wk_h = [load_weight_half(wkt, 0, split=True)]
            wv_h = []

            def emit_qt_prefetch0(sb):
                qt = pool_qt.tile([P, DI, SBW], F32R, tag="qt", name=f"qt{sb}")
                nc.gpsimd.dma_start(
                    qt[:],
                    qin.rearrange("(di p) s -> p di s", p=P)[
                        :, :, sb * SBW : (sb + 1) * SBW
                    ],
                )
                qt_tiles[sb] = qt

            def after_k0():
                prefetch_kt0(0, 4)
                nc.gpsimd.dma_start(ones_t[:], onesd[:])
                nc.gpsimd.dma_start(borep_t[:], borep[:])
                emit_qt_prefetch0(0)

            project_T(
                wk_h, None, kin, kT_d,
                split_first_tin=True,
                after_first_tin=lambda: (
                    nc.sync.dma_start(wv_t[:], wvec[:]),
                    wk_h.append(load_weight_half(wkt, 1)),
                ),
                after_cols=(
                    after_k0,
                    lambda: wv_h.append(load_weight_half(wvt, 0)),
                    lambda: (
                        wv_h.append(load_weight_half(wvt, 1)),
                        emit_qt_prefetch0(1),
                    ),
                ),
            )

            vr = vin.rearrange("(di p) s -> p di s", p=P)
            for jc4 in range(S // 512):
                tin = pool_in.tile([P, DI, 512], F32R, tag="tin")
                nc.gpsimd.dma_start(tin[:], vr[:, :, jc4 * 512 : (jc4 + 1) * 512])
                for jb in range(512 // P):
                    jg = jc4 * 4 + jb
                    for nn in range(D // 512):
                        ps = psum_p.tile([P, 512], F32, tag="ps")
                        for di in range(DI):
                            nc.tensor.matmul(
                                ps[:],
                                tin[:, di, jb * P : (jb + 1) * P],
                                wv_h[nn][:, di, :],
                                start=di == 0,
                                stop=di == DI - 1,
                            )
                        nc.vector.tensor_copy(
                            v_sb[:, jg, nn * 512 : (nn + 1) * 512], ps[:]
                        )

        with (
            tc.tile_pool(name="ptp", bufs=1) as pool_pt,
            tc.tile_pool(name="yp", bufs=4) as pool_y,
            tc.tile_pool(name="smal", bufs=2) as pool_small,
            tc.tile_pool(name="mskp", bufs=2) as pool_mask,
            tc.tile_pool(name="qkps", bufs=3, space="PSUM") as psum_qk,
            tc.tile_pool(name="lps", bufs=1, space="PSUM") as psum_l,
            tc.tile_pool(name="yps", bufs=4, space="PSUM") as psum_y,
        ):
            def emit_qt_prefetch(sb):
                qt = pool_qt.tile([P, DI, SBW], F32R, tag="qt", name=f"qt{sb}")
                nc.gpsimd.dma_start(
                    qt[:],
                    qin.rearrange("(di p) s -> p di s", p=P)[
                        :, :, sb * SBW : (sb + 1) * SBW
                    ],
                )
                qt_tiles[sb] = qt

            def emit_qk(sb):
                qt = qt_tiles[sb]
                pt = pool_pt.tile([P, JC, SBW], F32R, tag="pt", name=f"pt{sb}")
                for jc in range(nj):
                    if sb == 0 and jc < n_kt0:
                        kt = kt0_tiles[jc]
                    else:
                        kt = pool_kt.tile(
                            [P, DI, P], F32R, tag="kt", name=f"kt{sb}_{jc}"
                        )
                        nc.scalar.dma_start(
                            kt[:],
                            kT_d[:, :, jc * P : (jc + 1) * P].rearrange(
                                "di p j -> p di j"
                            ),
                        )
                    ps = psum_qk.tile([P, SBW], F32, tag="ps", name=f"qk{sb}_{jc}")
                    for di in range(DI):
                        nc.tensor.matmul(
                            ps[:],
                            kt[:, di, :],
                            qt[:, di, :],
                            start=di == 0,
                            stop=di == DI - 1,
                        )
                    nc.scalar.activation(
                        pt[:, jc, :],
                        ps[:],
                        mybir.ActivationFunctionType.Exp,
                        bias=wv_t[:, jc : jc + 1],
                    )
                    if variant == "masked":
                        mtile = pool_mask.tile([P, SBW], F32, tag="mt")
                        nc.sync.dma_start(
                            mtile[:],
                            maskt[jc * P : (jc + 1) * P, sb * SBW : (sb + 1) * SBW],
                        )
                        nc.vector.tensor_mul(pt[:, jc, :], pt[:, jc, :], mtile[:])
                return pt

            def emit_out(sb, pt):
                for ic in range(SBW // P):
                    njc = nj
                    l_ps = psum_l.tile([P, 32], F32, tag="lps", name=f"l{sb}_{ic}")
                    yps = [
                        psum_y.tile([P, 512], F32, tag="ypsum", name=f"y{sb}_{ic}_{dh}")
                        for dh in range(2)
                    ]
                    for jc in range(njc):
                        lhsT = pt[:, jc, ic * P : (ic + 1) * P]
                        for dh in range(2):
                            nc.tensor.matmul(
                                yps[dh][:],
                                lhsT,
                                v_sb[:, jc, dh * 512 : (dh + 1) * 512],
                                start=jc == 0,
                                stop=jc == njc - 1,
                            )
                        nc.tensor.matmul(
                            l_ps[:, :8],
                            lhsT,
                            ones_t[:, :8],
                            start=jc == 0,
                            stop=jc == njc - 1,
                        )
                    rinv = pool_small.tile([P, 1], F32, tag="rinv", name=f"ri{sb}_{ic}")
                    nc.vector.reciprocal(rinv[:], l_ps[:, 0:1])
                    for dh in range(2):
                        ysb = pool_y.tile(
                            [P, 512], F32, tag="y", name=f"ysb{sb}_{ic}_{dh}"
                        )
                        nc.scalar.mul(ysb[:], yps[dh][:], rinv[:])
                        nc.vector.tensor_add(
                            ysb[:], ysb[:], borep_t[:, dh * 512 : (dh + 1) * 512]
                        )
                        nc.sync.dma_start(
                            out[
                                sb * SBW + ic * P : sb * SBW + (ic + 1) * P,
                                dh * 512 : (dh + 1) * 512,
                            ],
                            ysb[:],
                        )

            for sb in range(NSB):
                pt = emit_qk(sb)
                emit_out(sb, pt)
                if sb + 2 < NSB:
                    emit_qt_prefetch(sb + 2)

    nc.compile()
    return nc


def _get_nc(variant: str):
    if variant not in _cache:
        if variant == "causal":
            _cache[variant] = _build_causal_fp8()
        else:
            _cache[variant] = _build_legacy(variant)
    return _cache[variant]


def _detect_variant(mask: np.ndarray) -> str:
    m = np.asarray(mask)[:, :, 0] != 0
    if m.all():
        return "full"
    if np.array_equal(m, np.tril(np.ones((S, S), dtype=bool))):
        return "causal"
    return "masked"


def _host_inputs(variant, query, key, value, mask, Wq, bq, Wk, bk, Wv, bv, Wo, bo, c):
    if variant == "causal":
        return _host_inputs_fp8(
            query, key, value, mask, Wq, bq, Wk, bk, Wv, bv, Wo, bo, c
        )
    bo_eff = (bo + Wo @ bv).astype(np.float32)
    m = {
        "qin": np.ascontiguousarray(query[:, c, :].T),
        "kin": np.ascontiguousarray(key[:, c, :].T),
        "vin": np.ascontiguousarray(value[:, c, :].T),
        "wkt": np.ascontiguousarray(
            (SCALE * (Wk.T.astype(np.float64) @ Wq.astype(np.float64))).astype(
                np.float32
            )
        ),
        "wvt": np.ascontiguousarray(
            (Wv.T.astype(np.float64) @ Wo.T.astype(np.float64)).astype(np.float32)
        ),
        "wvec": np.ascontiguousarray(
            (SCALE * (key[:, c, :] @ (Wk.T @ bq))).reshape(JC, P).T
        ),
        "borep": np.ascontiguousarray(np.broadcast_to(bo_eff, (P, D))),
        "onesd": np.ones((P, P), dtype=np.float32),
    }
    if variant == "masked":
        m["maskt"] = np.ascontiguousarray(
            (np.asarray(mask)[:, :, 0] != 0).T.astype(np.float32)
        )
    return m


def kernel(query, key, value, mask, Wq, bq, Wk, bk, Wv, bv, Wo, bo):
    query = np.asarray(query, dtype=np.float32)
    key = np.asarray(key, dtype=np.float32)
    value = np.asarray(value, dtype=np.float32)
    Wq = np.asarray(Wq, dtype=np.float32)
    Wk = np.asarray(Wk, dtype=np.float32)
    Wv = np.asarray(Wv, dtype=np.float32)
    Wo = np.asarray(Wo, dtype=np.float32)
    bq = np.asarray(bq, dtype=np.float32)
    bk = np.asarray(bk, dtype=np.float32)
    bv = np.asarray(bv, dtype=np.float32)
    bo = np.asarray(bo, dtype=np.float32)

    variant = _detect_variant(mask)
    nc = _get_nc(variant)
    in_maps = [
        _host_inputs(variant, query, key, value, mask, Wq, bq, Wk, bk, Wv, bv, Wo, bo, c)
        for c in CORES
    ]
    res = run_bass_kernel_spmd(nc, in_maps, core_ids=CORES)

    result = np.empty((S, B, D), dtype=np.float32)
    if variant == "causal":
        bo_eff = (bo + Wo @ bv).astype(np.float32)
        for c in CORES:
            o = res.results[c]["out"].astype(np.float32)
            if bo_eff.any():
                o = o + bo_eff
            result[:, c, :] = o
    else:
        for c in CORES:
            result[:, c, :] = res.results[c]["out"]
    return result


# revision 73
# speedup vs baseline: 1.0013x; 1.0013x over previous
"""Single-head causal attention (S=2048, B=8, D=1024) for 8 TRN2 NeuronCores.

Sharding: data-parallel over the batch dim - core c computes batch element c.

fp8 fast path (causal variant):
  - All heavy matmuls run as fp8(e4m3) DoubleRow: each instruction contracts
    256 rows (2x128 pair-interleaved) at 0.5 cycles per output column.
  - Host pre-quantizes q/k/v and the folded weights B = SCALE*Wk^T@Wq (scaled
    by SB_SCALE) and C = Wv^T@Wo^T (scaled by SC_SCALE) to fp8.
  - Accuracy: causal softmax rows with few keys amplify quantization noise,
    so a "clean" prefix covers the start of the sequence: K/V projections
    use 3-term hi/lo fp8 splits (hi*hi + lo*hi + hi*lo, ~fp16 accuracy at
    0.75x fp16's cost) for keys j < JCL (=128), and the attention for rows
    i < ICL (=128) runs in fp16; everything else is single-term fp8. exp()
    uses a global -CSHIFT shift (the denominator sums the same quantized
    weights, so the shift and all common-mode quantization cancel).
  - Softmax denominator rides the PV DoubleRow accumulation as an extra
    8-column all-(SC_SCALE) rhs; 1/l is applied in the epilogue (DVE muls,
    ACT only where exp is idle), output stored fp16.
  - G (projected keys) and v' (output-projected values) stay SBUF-resident;
    no DRAM scratch roundtrip. Dummy warmup matmuls bridge the first-DMA
    latency so the PE p-state ramp completes before real work.

Legacy fp32r path kept for non-causal masks.
"""

import math
from contextlib import ExitStack

import numpy as np
import ml_dtypes

import concourse.bass as bass
import concourse.mybir as mybir
import concourse.tile as tile
from concourse import bacc
from concourse.bass_utils import run_bass_kernel_spmd
from concourse.masks import make_identity

S, B, D = 2048, 8, 1024
P = 128
DI = D // P  # 8 contraction chunks
DI2 = D // 256  # 4 DoubleRow contraction chunks
JC = S // P  # 16 key chunks
NSB = 4  # query superblocks
SBW = S // NSB  # 512 queries per superblock
SCALE = 1.0 / math.sqrt(D)
CORES = list(range(8))
F32 = mybir.dt.float32
F32R = mybir.dt.float32r
F16 = mybir.dt.float16
F8 = mybir.dt.float8e4
NPF8 = ml_dtypes.float8_e4m3
DR = mybir.MatmulPerfMode.DoubleRow

SB_SCALE = 512.0  # B-weight scale (entries ~1e-3 would be subnormal in fp8)
SC_SCALE = 16.0  # C-weight scale
CSHIFT = 2.0  # global score shift before exp (keeps p in fp8 range)
CLEAN = 256  # attention rows < CLEAN run in fp16
CJC = CLEAN // P  # 2 attention-clean j-chunks
NF8 = S - CLEAN  # fp8 columns of q
JCL = 128  # projections computed cleanly (3-term) only for j < JCL
NF8K = S - JCL  # fp8 columns of k/v


_cache: dict[str, object] = {}


def _build_causal_fp8():
    nc = bacc.Bacc("TRN2", num_devices=len(CORES))

    qin8 = nc.dram_tensor("qin8", [P, DI2, 2, NF8], F8, kind="ExternalInput").ap()
    qin16 = nc.dram_tensor("qin16", [P, DI, CLEAN], F16, kind="ExternalInput").ap()
    kin8 = nc.dram_tensor("kin8", [P, DI2, 2, NF8K], F8, kind="ExternalInput").ap()
    kc8h = nc.dram_tensor("kc8h", [P, DI2, 2, JCL], F8, kind="ExternalInput").ap()
    kc8l = nc.dram_tensor("kc8l", [P, DI2, 2, JCL], F8, kind="ExternalInput").ap()
    vin8 = nc.dram_tensor("vin8", [P, DI2, 2, NF8K], F8, kind="ExternalInput").ap()
    vc8h = nc.dram_tensor("vc8h", [P, DI2, 2, JCL], F8, kind="ExternalInput").ap()
    vc8l = nc.dram_tensor("vc8l", [P, DI2, 2, JCL], F8, kind="ExternalInput").ap()
    wb8 = nc.dram_tensor("wb8", [P, DI2, 2, D], F8, kind="ExternalInput").ap()
    wb8l = nc.dram_tensor("wb8l", [P, DI2, 2, D], F8, kind="ExternalInput").ap()
    wc8 = nc.dram_tensor("wc8", [P, DI2, 2, D], F8, kind="ExternalInput").ap()
    wc8l = nc.dram_tensor("wc8l", [P, DI2, 2, D], F8, kind="ExternalInput").ap()
    wvec = nc.dram_tensor("wvec", [P, JC], F32, kind="ExternalInput").ap()
    out = nc.dram_tensor("out", [S, D], F16, kind="ExternalOutput").ap()

    with tile.TileContext(nc) as tc, ExitStack() as ctx:
        pool_const = ctx.enter_context(tc.tile_pool(name="const", bufs=1))
        pool_g = ctx.enter_context(tc.tile_pool(name="gres", bufs=1))
        pool_v = ctx.enter_context(tc.tile_pool(name="vres", bufs=1))
        pool_q = ctx.enter_context(tc.tile_pool(name="qres", bufs=1))
        pool_pt0 = ctx.enter_context(tc.tile_pool(name="pt0", bufs=1))
        pool_y = ctx.enter_context(tc.tile_pool(name="yp", bufs=4))
        pool_small = ctx.enter_context(tc.tile_pool(name="smal", bufs=4))

        wv_t = pool_const.tile([P, JC], F32)
        ones8_t = pool_const.tile([P, 2, 8], F8)
        ones16_t = pool_const.tile([P, 8], F16)
        garb_t = pool_const.tile([P, P], F16)  # never written: PE warmup fuel

        g8 = pool_g.tile([P, DI, S], F8)
        g16 = pool_g.tile([P, DI, JCL], F16)
        v8 = pool_v.tile([P, JC, D], F8)
        v16 = pool_v.tile([P, 1, D], F16)
        q8 = pool_q.tile([P, DI2, 2, NF8], F8)
        q16 = pool_q.tile([P, DI, CLEAN], F16)

        # cast-engine alternation
        _ce = [1]

        def cast(dst, src, force=None):
            e = force if force is not None else ("dve" if _ce[0] % 2 == 0 else "act")
            if e == "dve":
                nc.vector.tensor_copy(dst, src)
            else:
                nc.scalar.copy(dst, src)
            _ce[0] += 1


        def emit_epilogue(sb, icl, l_ps, yps, last=False):
            rinv = pool_small.tile([P, 1], F32, tag="ri", name=f"ri{sb}_{icl}")
            nc.vector.reciprocal(rinv[:], l_ps[:, 0:1])
            r0 = sb * SBW + icl * P
            if last:
                # final tile: both halves in parallel on separate engines
                # and queues to shorten the kernel tail
                ysb = pool_y.tile([P, D], F16, tag="y", name=f"y{sb}_{icl}")
                nc.vector.tensor_scalar_mul(ysb[:, :SBW], yps[0][:], rinv[:, 0:1])
                nc.scalar.mul(ysb[:, SBW:], yps[1][:], rinv[:, 0:1])
                nc.sync.dma_start(out[r0 : r0 + P, :SBW], ysb[:, :SBW])
                nc.scalar.dma_start(out[r0 : r0 + P, SBW:], ysb[:, SBW:])
            else:
                ysb = pool_y.tile([P, D], F16, tag="y", name=f"y{sb}_{icl}")
                nc.vector.tensor_scalar_mul(ysb[:, :SBW], yps[0][:], rinv[:, 0:1])
                if sb == NSB - 1:
                    # ACT is exp-free during sb3 PV; share the mul load
                    nc.scalar.mul(ysb[:, SBW:], yps[1][:], rinv[:, 0:1])
                else:
                    # keep ACT free for exp: both muls on DVE
                    nc.vector.tensor_scalar_mul(
                        ysb[:, SBW:], yps[1][:], rinv[:, 0:1]
                    )
                nc.sync.dma_start(out[r0 : r0 + P, :], ysb[:])

        # ---------------- phase A: projections ----------------
        with (
            tc.tile_pool(name="wts", bufs=1) as pool_w,
            tc.tile_pool(name="ins", bufs=1) as pool_in,
            tc.tile_pool(name="pps", bufs=8, space="PSUM") as psum_a,
        ):
            kin8_t = pool_in.tile([P, DI2, 2, NF8K], F8, name="kin8")
            kc8h_t = pool_in.tile([P, DI2, 2, JCL], F8, name="kc8h")
            kc8l_t = pool_in.tile([P, DI2, 2, JCL], F8, name="kc8l")
            vin8_t = pool_in.tile([P, DI2, 2, NF8K], F8, name="vin8")
            vc8h_t = pool_in.tile([P, DI2, 2, JCL], F8, name="vc8h")
            vc8l_t = pool_in.tile([P, DI2, 2, JCL], F8, name="vc8l")
            wb8_t = pool_w.tile([P, DI2, 2, D], F8, name="wb8")
            wb8l_t = pool_w.tile([P, DI2, 2, D], F8, name="wb8l")
            wc8_t = pool_w.tile([P, DI2, 2, D], F8, name="wc8")
            wc8l_t = pool_w.tile([P, DI2, 2, D], F8, name="wc8l")

            # scalar queue: wb8 per-md chunks (K-clean consumes md-outer),
            # then wc8; ACT is cast-free until ~6us
            nc.scalar.dma_start(wb8_t[:, 0, :, :SBW], wb8[:, 0, :, :SBW])
            nc.scalar.dma_start(wb8_t[:, 0, :, SBW:], wb8[:, 0, :, SBW:])
            for md in range(1, DI2):
                nc.scalar.dma_start(wb8_t[:, md], wb8[:, md])
            # sync queue: small clean-prefix hi/lo inputs + wc8 + q16 + wvec
            nc.sync.dma_start(kc8h_t[:], kc8h[:])
            nc.sync.dma_start(kc8l_t[:], kc8l[:])
            nc.sync.dma_start(vc8h_t[:], vc8h[:])
            nc.sync.dma_start(vc8l_t[:], vc8l[:])
            for md in range(DI2):
                nc.sync.dma_start(wc8_t[:, md], wc8[:, md])
            nc.sync.dma_start(q16[:], qin16[:])
            nc.sync.dma_start(wv_t[:], wvec[:])

            # gpsimd queue: lo-weights per-md first (K-clean term 3), then
            # kin8 chunks in the jb-outer consumption order
            kblocks = [(128, 640), (640, 1152), (1152, 1664),
                       (1664, 1920), (0, 128)]
            nc.gpsimd.memset(ones8_t[:], SC_SCALE)
            nc.gpsimd.memset(ones16_t[:], SC_SCALE)
            for md in range(DI2):
                nc.gpsimd.dma_start(wb8l_t[:, md], wb8l[:, md])
            nc.gpsimd.dma_start(kin8_t[:, :, :, 128:640], kin8[:, :, :, 128:640])
            nc.gpsimd.dma_start(wc8l_t[:], wc8l[:])
            for lo, hi in kblocks[1:]:
                nc.gpsimd.dma_start(kin8_t[:, :, :, lo:hi], kin8[:, :, :, lo:hi])
            HN = NF8 // 2  # 896
            nc.gpsimd.dma_start(vin8_t[:, :, :, :HN], vin8[:, :, :, :HN])
            nc.gpsimd.dma_start(vin8_t[:, :, :, HN:], vin8[:, :, :, HN:])
            nc.gpsimd.dma_start(q8[:, :, :, :HN], qin8[:, :, :, :HN])
            nc.gpsimd.dma_start(q8[:, :, :, HN:], qin8[:, :, :, HN:])

            # PE warmup: dummy matmuls on an uninitialized (all-zero) tile
            # bridge the first-DMA latency (~3us) and keep the PE p-state
            # ramp counting so the real work starts at full clock.
            warm_ps = psum_a.tile([P, SBW], F32, tag="ps", name="warm")
            nc.vector.memset(garb_t[:], 0.0)
            for _ in range(4):
                nc.tensor.matmul(
                    warm_ps[:, :64], garb_t[:], garb_t[:, :64], start=True,
                    stop=True,
                )

            # K-proj clean: G^T[m-chunk, j<JCL] via 3-term hi/lo fp8 DR
            # (hi*hi + lo_k*hi + hi*lo_B: ~fp16 accuracy at 0.75x the cost).
            # md-outer so the first matmuls only need the first wb8 chunk.
            for mh in range(2):
                kc_ps = [
                    psum_a.tile([P, SBW], F32, tag="ps", name=f"kc{mh}_{m4}")
                    for m4 in range(4)
                ]
                for md in range(DI2):
                    for m4 in range(4):
                        m = 4 * mh + m4
                        for t, (wt, kt) in enumerate(
                            ((wb8_t, kc8h_t), (wb8_t, kc8l_t), (wb8l_t, kc8h_t))
                        ):
                            nc.tensor.matmul(
                                kc_ps[m4][:, :JCL],
                                wt[:, md, :, m * P : (m + 1) * P],
                                kt[:, md],
                                start=md == 0 and t == 0,
                                stop=md == DI2 - 1 and t == 2,
                                perf_mode=DR,
                            )
                for m4 in range(4):
                    m = 4 * mh + m4
                    cast(g16[:, m, :], kc_ps[m4][:, :JCL])
                    cast(g8[:, m, :JCL], kc_ps[m4][:, :JCL])

            # K-proj fp8: j >= JCL (kin8 col x = j - JCL)
            # jb-outer so each j-block only needs its own kin8 chunk
            for lo, hi in kblocks:
                for m in range(DI):
                    w = hi - lo
                    ps = psum_a.tile([P, SBW], F32, tag="ps", name=f"kf{m}_{lo}")
                    for md in range(DI2):
                        nc.tensor.matmul(
                            ps[:, :w],
                            wb8_t[:, md, :, m * P : (m + 1) * P],
                            kin8_t[:, md, :, lo:hi],
                            start=md == 0,
                            stop=md == DI2 - 1,
                            perf_mode=DR,
                        )
                    cast(g8[:, m, JCL + lo : JCL + hi], ps[:, :w])

            # V-proj clean: v'[j<JCL, :] via 3-term hi/lo fp8 DR
            vc_ps = [
                psum_a.tile([P, SBW], F32, tag="ps", name=f"vc{je}")
                for je in range(2)
            ]
            for md in range(DI2):
                for jcl in range(1):
                    for eh in range(2):
                        for t, (vt, wt) in enumerate(
                            ((vc8h_t, wc8_t), (vc8l_t, wc8_t), (vc8h_t, wc8l_t))
                        ):
                            nc.tensor.matmul(
                                vc_ps[2 * jcl + eh][:],
                                vt[:, md, :, jcl * P : (jcl + 1) * P],
                                wt[:, md, :, eh * SBW : (eh + 1) * SBW],
                                start=md == 0 and t == 0,
                                stop=md == DI2 - 1 and t == 2,
                                perf_mode=DR,
                            )
            for jcl in range(1):
                for eh in range(2):
                    ps = vc_ps[2 * jcl + eh]
                    cast(v16[:, jcl, eh * SBW : (eh + 1) * SBW], ps[:])
                    cast(v8[:, jcl, eh * SBW : (eh + 1) * SBW], ps[:])

            # V-proj fp8: j >= JCL (vin8 col x = j - JCL)
            for jc in range(1, JC):
                for eh in range(2):
                    ps = psum_a.tile([P, SBW], F32, tag="ps", name=f"vf{jc}_{eh}")
                    for md in range(DI2):
                        nc.tensor.matmul(
                            ps[:],
                            vin8_t[:, md, :, jc * P - JCL : (jc + 1) * P - JCL],
                            wc8_t[:, md, :, eh * SBW : (eh + 1) * SBW],
                            start=md == 0,
                            stop=md == DI2 - 1,
                            perf_mode=DR,
                        )
                    cast(v8[:, jc, eh * SBW : (eh + 1) * SBW], ps[:])

        # ---------------- phase B: attention ----------------
        with (
            tc.tile_pool(name="ptp", bufs=2) as pool_pt,
            tc.tile_pool(name="yps", bufs=5, space="PSUM") as psum_y,
            tc.tile_pool(name="qkps", bufs=3, space="PSUM") as psum_qk,
        ):

            # ---- sb0 clean part: rows [0, ICL) in fp16
            pt16 = pool_pt0.tile([P, CJC, CLEAN], F16, name="pt16")
            for jc in range(CJC):
                off = jc * P
                ps = psum_qk.tile([P, SBW], F32, tag="qk", name=f"qkc{jc}")
                for di in range(DI):
                    lh = (g16[:, di, :] if jc == 0
                          else g8[:, di, jc * P : (jc + 1) * P])
                    nc.tensor.matmul(
                        ps[:, off:CLEAN],
                        lh,
                        q16[:, di, off:],
                        start=di == 0,
                        stop=di == DI - 1,
                    )
                nc.scalar.activation(
                    pt16[:, jc, off:],
                    ps[:, off:CLEAN],
                    mybir.ActivationFunctionType.Exp,
                    bias=wv_t[:, jc : jc + 1],
                    scale=1.0 / SB_SCALE,
                )
                nc.gpsimd.affine_select(
                    out=pt16[:, jc, off : off + P],
                    in_=pt16[:, jc, off : off + P],
                    compare_op=mybir.AluOpType.is_ge,
                    fill=0.0,
                    base=0,
                    pattern=[[1, P]],
                    channel_multiplier=-1,
                )
            for icl in range(CJC):
                l_ps = psum_y.tile([P, SBW], F32, tag="yp", name=f"l0_{icl}")
                yps = [
                    psum_y.tile([P, SBW], F32, tag="yp", name=f"yp0_{icl}_{eh}")
                    for eh in range(2)
                ]
                for jc in range(icl + 1):
                    lhsT = pt16[:, jc, icl * P : (icl + 1) * P]
                    for eh in range(2):
                        rh = (v16[:, 0, eh * SBW : (eh + 1) * SBW] if jc == 0
                              else v8[:, jc, eh * SBW : (eh + 1) * SBW])
                        nc.tensor.matmul(
                            yps[eh][:],
                            lhsT,
                            rh,
                            start=jc == 0,
                            stop=jc == icl,
                        )
                    nc.tensor.matmul(
                        l_ps[:, :1],
                        lhsT,
                        ones16_t[:, 0:1],
                        start=jc == 0,
                        stop=jc == icl,
                    )
                emit_epilogue(0, icl, l_ps, yps)

            # ---- sb0 fp8 part: rows [ICL, SBW) (q8 col x = i - ICL)
            pt8a = pool_pt0.tile([P, 4, 256], F8, name="pt8a")
            nc.gpsimd.memset(pt8a[:, 3, 0:P], 0.0)
            for jc in range(4):
                off = max(0, jc * P - CLEAN)  # local offset in [0,256)
                ps = psum_qk.tile([P, SBW], F32, tag="qk", name=f"qka{jc}")
                for md in range(DI2):
                    nc.tensor.matmul(
                        ps[:, off:256],
                        g8[:, 2 * md : 2 * md + 2, jc * P : (jc + 1) * P],
                        q8[:, md, :, off:256],
                        start=md == 0,
                        stop=md == DI2 - 1,
                        perf_mode=DR,
                    )
                nc.scalar.activation(
                    pt8a[:, jc, off:],
                    ps[:, off:256],
                    mybir.ActivationFunctionType.Exp,
                    bias=wv_t[:, jc : jc + 1],
                    scale=1.0 / SB_SCALE,
                )
                if jc * P >= CLEAN:
                    nc.gpsimd.affine_select(
                        out=pt8a[:, jc, off : off + P],
                        in_=pt8a[:, jc, off : off + P],
                        compare_op=mybir.AluOpType.is_ge,
                        fill=0.0,
                        base=CLEAN + off - jc * P,
                        pattern=[[1, P]],
                        channel_multiplier=-1,
                    )
            for icl in (2, 3):
                loc = icl * P - CLEAN
                l_ps = psum_y.tile([P, SBW], F32, tag="yp", name=f"l0a{icl}")
                yps = [
                    psum_y.tile([P, SBW], F32, tag="yp", name=f"yp0a{icl}_{eh}")
                    for eh in range(2)
                ]
                npair = icl // 2 + 1
                for m in range(npair):
                    lhsT = pt8a[:, 2 * m : 2 * m + 2, loc : loc + P]
                    for eh in range(2):
                        nc.tensor.matmul(
                            yps[eh][:],
                            lhsT,
                            v8[:, 2 * m : 2 * m + 2, eh * SBW : (eh + 1) * SBW],
                            start=m == 0,
                            stop=m == npair - 1,
                            perf_mode=DR,
                        )
                    nc.tensor.matmul(
                        l_ps[:, :1],
                        lhsT,
                        ones8_t[:, :, 0:1],
                        start=m == 0,
                        stop=m == npair - 1,
                        perf_mode=DR,
                    )
                emit_epilogue(0, icl, l_ps, yps)

            # ---- sb1..3: pure fp8. Two pt buffers reused manually
            # (sb3 reuses sb1's tile) so the diagonal-region memsets for
            # both rounds run once, early, off the Pool critical path.
            pt_bufs = {}
            for sb in range(1, NSB):
                nj = 4 * sb + 4
                if sb - 2 in pt_bufs:
                    pt = pt_bufs[sb - 2]
                else:
                    pt = pool_pt.tile([P, JC, SBW], F8, tag="pt", name=f"pt{sb}")
                    pt_bufs[sb] = pt
                    for s2 in (sb, sb + 2):
                        if s2 < NSB:
                            for t in (1, 2, 3):
                                nc.gpsimd.memset(
                                    pt[:, 4 * s2 + t, 0 : t * P], 0.0
                                )
                for jc in range(nj):
                    off = max(0, (jc - 4 * sb) * P)
                    ps = psum_qk.tile([P, SBW], F32, tag="qk", name=f"qk{sb}_{jc}")
                    for md in range(DI2):
                        nc.tensor.matmul(
                            ps[:, off:],
                            g8[:, 2 * md : 2 * md + 2, jc * P : (jc + 1) * P],
                            q8[:, md, :, sb * SBW + off - CLEAN : (sb + 1) * SBW - CLEAN],
                            start=md == 0,
                            stop=md == DI2 - 1,
                            perf_mode=DR,
                        )
                    nc.scalar.activation(
                        pt[:, jc, off:],
                        ps[:, off:],
                        mybir.ActivationFunctionType.Exp,
                        bias=wv_t[:, jc : jc + 1],
                        scale=1.0 / SB_SCALE,
                    )
                    if jc >= 4 * sb:
                        bend = min(off + P, SBW)
                        nc.gpsimd.affine_select(
                            out=pt[:, jc, off:bend],
                            in_=pt[:, jc, off:bend],
                            compare_op=mybir.AluOpType.is_ge,
                            fill=0.0,
                            base=sb * SBW + off - jc * P,
                            pattern=[[1, bend - off]],
                            channel_multiplier=-1,
                        )
                for icl in range(4):
                    ic = 4 * sb + icl
                    npair = ic // 2 + 1
                    l_ps = psum_y.tile([P, SBW], F32, tag="yp", name=f"l{sb}_{icl}")
                    yps = [
                        psum_y.tile([P, SBW], F32, tag="yp", name=f"yp{sb}_{icl}_{eh}")
                        for eh in range(2)
                    ]
                    for m in range(npair):
                        lhsT = pt[:, 2 * m : 2 * m + 2, icl * P : (icl + 1) * P]
                        for eh in range(2):
                            nc.tensor.matmul(
                                yps[eh][:],
                                lhsT,
                                v8[:, 2 * m : 2 * m + 2, eh * SBW : (eh + 1) * SBW],
                                start=m == 0,
                                stop=m == npair - 1,
                                perf_mode=DR,
                            )
                        nc.tensor.matmul(
                            l_ps[:, :1],
                            lhsT,
                            ones8_t[:, :, 0:1],
                            start=m == 0,
                            stop=m == npair - 1,
                            perf_mode=DR,
                        )
                    emit_epilogue(
                        sb, icl, l_ps, yps, last=(sb == NSB - 1 and icl == 3)
                    )

    nc.compile()
    return nc


def _host_inputs_fp8(query, key, value, mask, Wq, bq, Wk, bk, Wv, bv, Wo, bo, c):
    q = np.ascontiguousarray(query[:, c, :])
    k = np.ascontiguousarray(key[:, c, :])
    v = np.ascontiguousarray(value[:, c, :])
    Bm = (
        SCALE * SB_SCALE * (Wk.T.astype(np.float64) @ Wq.astype(np.float64))
    ).astype(np.float32)
    Cm = (
        SC_SCALE * (Wv.T.astype(np.float64) @ Wo.T.astype(np.float64))
    ).astype(np.float32)

    def drl(xT):  # [D, N] -> [P, DI2, 2, N] fp32 pair-interleave layout
        n = xT.shape[1]
        return np.ascontiguousarray(xT.reshape(DI2, 2, P, n).transpose(2, 0, 1, 3))

    def dr8(xT):  # fp8 (hi) in DR layout
        return drl(xT).astype(NPF8)

    def dr8hl(xT):  # (hi, lo) fp8 pair in DR layout
        a = drl(xT)
        hi = a.astype(NPF8)
        lo = (a - hi.astype(np.float32)).astype(NPF8)
        return hi, lo

    def pl16(xT, n):  # [D, N] -> [P, DI, n] fp16
        return np.ascontiguousarray(
            xT[:, :n].reshape(DI, P, n).transpose(1, 0, 2)
        ).astype(np.float16)

    qT, kT, vT = q.T, k.T, v.T
    wvec = (SCALE * (k @ (Wk.T @ bq)) - CSHIFT).astype(np.float32)
    kc8h, kc8l = dr8hl(np.ascontiguousarray(kT[:, :JCL]))
    vc8h, vc8l = dr8hl(np.ascontiguousarray(vT[:, :JCL]))
    wb8, wb8l = dr8hl(Bm)
    wc8, wc8l = dr8hl(Cm)
    return {
        "qin8": dr8(np.ascontiguousarray(qT[:, CLEAN:])),
        "qin16": pl16(qT, CLEAN),
        "kin8": dr8(np.ascontiguousarray(kT[:, JCL:])),
        "kc8h": kc8h,
        "kc8l": kc8l,
        "vin8": dr8(np.ascontiguousarray(vT[:, JCL:])),
        "vc8h": vc8h,
        "vc8l": vc8l,
        "wb8": wb8,
        "wb8l": wb8l,
        "wc8": wc8,
        "wc8l": wc8l,
        "wvec": np.ascontiguousarray(wvec.reshape(JC, P).T),
    }


# ---------------------------------------------------------------------------
# legacy fp32r kernel (fallback for non-causal masks)
# ---------------------------------------------------------------------------


def _build_legacy(variant: str):
    """variant: 'full' (no mask), 'masked' (arbitrary 0/1 mask from DRAM)."""
    assert variant in ("full", "masked")
    nc = bacc.Bacc("TRN2", num_devices=len(CORES))

    qin = nc.dram_tensor("qin", [D, S], F32R, kind="ExternalInput").ap()
    kin = nc.dram_tensor("kin", [D, S], F32R, kind="ExternalInput").ap()
    vin = nc.dram_tensor("vin", [D, S], F32R, kind="ExternalInput").ap()
    wkt = nc.dram_tensor("wkt", [D, D], F32R, kind="ExternalInput").ap()
    wvt = nc.dram_tensor("wvt", [D, D], F32R, kind="ExternalInput").ap()
    wvec = nc.dram_tensor("wvec", [P, JC], F32, kind="ExternalInput").ap()
    borep = nc.dram_tensor("borep", [P, D], F32, kind="ExternalInput").ap()
    onesd = nc.dram_tensor("onesd", [P, P], F32R, kind="ExternalInput").ap()
    if variant == "masked":
        maskt = nc.dram_tensor("maskt", [S, S], F32, kind="ExternalInput").ap()
    out = nc.dram_tensor("out", [S, D], F32, kind="ExternalOutput").ap()

    kT_d = nc.dram_tensor("kT_d", [DI, P, S], F32R).ap()

    nj = JC

    with tile.TileContext(nc) as tc, ExitStack() as ctx:
        pool_const = ctx.enter_context(tc.tile_pool(name="const", bufs=1))
        pool_v = ctx.enter_context(tc.tile_pool(name="vres", bufs=1))
        pool_qt = ctx.enter_context(tc.tile_pool(name="qtp", bufs=2))
        pool_kt = ctx.enter_context(tc.tile_pool(name="ktp", bufs=3))

        ident = pool_const.tile([P, P], F32)
        make_identity(nc, ident[:])
        ones_t = pool_const.tile([P, P], F32R)
        wv_t = pool_const.tile([P, JC], F32)
        borep_t = pool_const.tile([P, D], F32)

        v_sb = pool_v.tile([P, JC, D], F32R)

        qt_tiles = {}
        n_kt0 = 3
        kt0_tiles = [
            pool_kt.tile([P, DI, P], F32R, tag="kt", name=f"kt0_{jc}")
            for jc in range(n_kt0)
        ]

        with (
            tc.tile_pool(name="wts", bufs=3) as pool_w,
            tc.tile_pool(name="ins", bufs=2) as pool_in,
            tc.tile_pool(name="stg", bufs=4) as pool_stage,
            tc.tile_pool(name="pps", bufs=4, space="PSUM") as psum_p,
        ):

            def load_weight_half(w_dram, h, split=False):
                wr = w_dram.rearrange("(di p) o -> p di o", p=P)
                wt = pool_w.tile([P, DI, 512], F32R, tag="wt", name=f"w{h}")
                if split:
                    for m in range(4):
                        nc.sync.dma_start(
                            wt[:, :, m * P : (m + 1) * P],
                            wr[:, :, h * 512 + m * P : h * 512 + (m + 1) * P],
                        )
                else:
                    nc.scalar.dma_start(wt[:], wr[:, :, h * 512 : (h + 1) * 512])
                return wt

            def wslice(halves, di, m):
                return halves[m // 4][:, di, (m % 4) * P : (m % 4 + 1) * P]

            def project_T(w_halves, b_tile, x_dram, dst_dram, split_first_tin=False,
                          after_cols=(), after_first_tin=None):
                xr = x_dram.rearrange("(di p) s -> p di s", p=P)
                for jc4 in range(S // 512):
                    tin = pool_in.tile([P, DI, 512], F32R, tag="tin")
                    if jc4 == 0 and split_first_tin:
                        for di in range(DI):
                            nc.gpsimd.dma_start(tin[:, di, :], xr[:, di, 0:512])
                    else:
                        nc.sync.dma_start(
                            tin[:], xr[:, :, jc4 * 512 : (jc4 + 1) * 512]
                        )
                    if jc4 == 0 and after_first_tin is not None:
                        after_first_tin()
                    for m in range(DI):
                        ps = psum_p.tile([P, 512], F32, tag="ps")
                        for di in range(DI):
                            nc.tensor.matmul(
                                ps[:],
                                wslice(w_halves, di, m),
                                tin[:, di, :],
                                start=di == 0,
                                stop=di == DI - 1,
                            )
                        st = pool_stage.tile([P, 512], F32R, tag="st")
                        if b_tile is None:
                            nc.vector.tensor_copy(st[:], ps[:])
                        else:
                            nc.vector.tensor_scalar_add(
                                st[:], ps[:], b_tile[:, m : m + 1]
                            )
                        nc.scalar.dma_start(
                            dst_dram[m, :, jc4 * 512 : (jc4 + 1) * 512], st[:]
                        )
                    if after_cols and jc4 < len(after_cols) and after_cols[jc4]:
                        after_cols[jc4]()

            def prefetch_kt0(a, b):
                for jc in range(a, min(b, n_kt0)):
                    nc.gpsimd.dma_start(
                        kt0_tiles[jc][:],
                        kT_d[:, :, jc * P : (jc + 1) * P].rearrange(
                            "di p j -> p di j"
                        ),
                    )

            wk_h = [load_weight_half(wkt, 0, split=True)]
            wv_h = []

            def emit_qt_prefetch0(sb):
                qt = pool_qt.tile([P, DI, SBW], F32R, tag="qt", name=f"qt{sb}")
                nc.gpsimd.dma_start(
                    qt[:],
                    qin.rearrange("(di p) s -> p di s", p=P)[
                        :, :, sb * SBW : (sb + 1) * SBW
                    ],
                )
                qt_tiles[sb] = qt

            def after_k0():
                prefetch_kt0(0, 4)
                nc.gpsimd.dma_start(ones_t[:], onesd[:])
                nc.gpsimd.dma_start(borep_t[:], borep[:])
                emit_qt_prefetch0(0)

            project_T(
                wk_h, None, kin, kT_d,
                split_first_tin=True,
                after_first_tin=lambda: (
                    nc.sync.dma_start(wv_t[:], wvec[:]),
                    wk_h.append(load_weight_half(wkt, 1)),
                ),
                after_cols=(
                    after_k0,
                    lambda: wv_h.append(load_weight_half(wvt, 0)),
                    lambda: (
                        wv_h.append(load_weight_half(wvt, 1)),
                        emit_qt_prefetch0(1),
                    ),
                ),
            )

            vr = vin.rearrange("(di p) s -> p di s", p=P)
            for jc4 in range(S // 512):
                tin = pool_in.tile([P, DI, 512], F32R, tag="tin")
                nc.gpsimd.dma_start(tin[:], vr[:, :, jc4 * 512 : (jc4 + 1) * 512])
                for jb in range(512 // P):
                    jg = jc4 * 4 + jb
                    for nn in range(D // 512):
                        ps = psum_p.tile([P, 512], F32, tag="ps")
                        for di in range(DI):
                            nc.tensor.matmul(
                                ps[:],
                                tin[:, di, jb * P : (jb + 1) * P],
                                wv_h[nn][:, di, :],
                                start=di == 0,
                                stop=di == DI - 1,
                            )
                        nc.vector.tensor_copy(
                            v_sb[:, jg, nn * 512 : (nn + 1) * 512], ps[:]
                        )

        with (
            tc.tile_pool(name="ptp", bufs=1) as pool_pt,
            tc.tile_pool(name="yp", bufs=4) as pool_y,
            tc.tile_pool(name="smal", bufs=2) as pool_small,
            tc.tile_pool(name="mskp", bufs=2) as pool_mask,
            tc.tile_pool(name="qkps", bufs=3, space="PSUM") as psum_qk,
            tc.tile_pool(name="lps", bufs=1, space="PSUM") as psum_l,
            tc.tile_pool(name="yps", bufs=4, space="PSUM") as psum_y,
        ):
            def emit_qt_prefetch(sb):
                qt = pool_qt.tile([P, DI, SBW], F32R, tag="qt", name=f"qt{sb}")
                nc.gpsimd.dma_start(
                    qt[:],
                    qin.rearrange("(di p) s -> p di s", p=P)[
                        :, :, sb * SBW : (sb + 1) * SBW
                    ],
                )
                qt_tiles[sb] = qt

            def emit_qk(sb):
                qt = qt_tiles[sb]
                pt = pool_pt.tile([P, JC, SBW], F32R, tag="pt", name=f"pt{sb}")
                for jc in range(nj):
                    if sb == 0 and jc < n_kt0:
                        kt = kt0_tiles[jc]
                    else:
                        kt = pool_kt.tile(
                            [P, DI, P], F32R, tag="kt", name=f"kt{sb}_{jc}"
                        )
                        nc.scalar.dma_start(
                            kt[:],
                            kT_d[:, :, jc * P : (jc + 1) * P].rearrange(
                                "di p j -> p di j"
                            ),
                        )
                    ps = psum_qk.tile([P, SBW], F32, tag="ps", name=f"qk{sb}_{jc}")
                    for di in range(DI):
                        nc.tensor.matmul(
                            ps[:],
                            kt[:, di, :],
                            qt[:, di, :],
                            start=di == 0,
                            stop=di == DI - 1,
                        )
                    nc.scalar.activation(
                        pt[:, jc, :],
                        ps[:],
                        mybir.ActivationFunctionType.Exp,
                        bias=wv_t[:, jc : jc + 1],
                    )
                    if variant == "masked":
                        mtile = pool_mask.tile([P, SBW], F32, tag="mt")
                        nc.sync.dma_start(
                            mtile[:],
                            maskt[jc * P : (jc + 1) * P, sb * SBW : (sb + 1) * SBW],
                        )
                        nc.vector.tensor_mul(pt[:, jc, :], pt[:, jc, :], mtile[:])
                return pt

            def emit_out(sb, pt):
                for ic in range(SBW // P):
                    njc = nj
                    l_ps = psum_l.tile([P, 32], F32, tag="lps", name=f"l{sb}_{ic}")
                    yps = [
                        psum_y.tile([P, 512], F32, tag="ypsum", name=f"y{sb}_{ic}_{dh}")
                        for dh in range(2)
                    ]
                    for jc in range(njc):
                        lhsT = pt[:, jc, ic * P : (ic + 1) * P]
                        for dh in range(2):
                            nc.tensor.matmul(
                                yps[dh][:],
                                lhsT,
                                v_sb[:, jc, dh * 512 : (dh + 1) * 512],
                                start=jc == 0,
                                stop=jc == njc - 1,
                            )
                        nc.tensor.matmul(
                            l_ps[:, :8],
                            lhsT,
                            ones_t[:, :8],
                            start=jc == 0,
                            stop=jc == njc - 1,
                        )
                    rinv = pool_small.tile([P, 1], F32, tag="rinv", name=f"ri{sb}_{ic}")
                    nc.vector.reciprocal(rinv[:], l_ps[:, 0:1])
                    for dh in range(2):
                        ysb = pool_y.tile(
                            [P, 512], F32, tag="y", name=f"ysb{sb}_{ic}_{dh}"
                        )
                        nc.scalar.mul(ysb[:], yps[dh][:], rinv[:])
                        nc.vector.tensor_add(
                            ysb[:], ysb[:], borep_t[:, dh * 512 : (dh + 1) * 512]
                        )
                        nc.sync.dma_start(
                            out[
                                sb * SBW + ic * P : sb * SBW + (ic + 1) * P,
                                dh * 512 : (dh + 1) * 512,
                            ],
                            ysb[:],
                        )

            for sb in range(NSB):
                pt = emit_qk(sb)
                emit_out(sb, pt)
                if sb + 2 < NSB:
                    emit_qt_prefetch(sb + 2)

    nc.compile()
    return nc


def _get_nc(variant: str):
    if variant not in _cache:
        if variant == "causal":
            _cache[variant] = _build_causal_fp8()
        else:
            _cache[variant] = _build_legacy(variant)
    return _cache[variant]


def _detect_variant(mask: np.ndarray) -> str:
    m = np.asarray(mask)[:, :, 0] != 0
    if m.all():
        return "full"
    if np.array_equal(m, np.tril(np.ones((S, S), dtype=bool))):
        return "causal"
    return "masked"


def _host_inputs(variant, query, key, value, mask, Wq, bq, Wk, bk, Wv, bv, Wo, bo, c):
    if variant == "causal":
        return _host_inputs_fp8(
            query, key, value, mask, Wq, bq, Wk, bk, Wv, bv, Wo, bo, c
        )
    bo_eff = (bo + Wo @ bv).astype(np.float32)
    m = {
        "qin": np.ascontiguousarray(query[:, c, :].T),
        "kin": np.ascontiguousarray(key[:, c, :].T),
        "vin": np.ascontiguousarray(value[:, c, :].T),
        "wkt": np.ascontiguousarray(
            (SCALE * (Wk.T.astype(np.float64) @ Wq.astype(np.float64))).astype(
                np.float32
            )
        ),
        "wvt": np.ascontiguousarray(
            (Wv.T.astype(np.float64) @ Wo.T.astype(np.float64)).astype(np.float32)
        ),
        "wvec": np.ascontiguousarray(
            (SCALE * (key[:, c, :] @ (Wk.T @ bq))).reshape(JC, P).T
        ),
        "borep": np.ascontiguousarray(np.broadcast_to(bo_eff, (P, D))),
        "onesd": np.ones((P, P), dtype=np.float32),
    }
    if variant == "masked":
        m["maskt"] = np.ascontiguousarray(
            (np.asarray(mask)[:, :, 0] != 0).T.astype(np.float32)
        )
    return m


def kernel(query, key, value, mask, Wq, bq, Wk, bk, Wv, bv, Wo, bo):
    query = np.asarray(query, dtype=np.float32)
    key = np.asarray(key, dtype=np.float32)
    value = np.asarray(value, dtype=np.float32)
    Wq = np.asarray(Wq, dtype=np.float32)
    Wk = np.asarray(Wk, dtype=np.float32)
    Wv = np.asarray(Wv, dtype=np.float32)
    Wo = np.asarray(Wo, dtype=np.float32)
    bq = np.asarray(bq, dtype=np.float32)
    bk = np.asarray(bk, dtype=np.float32)
    bv = np.asarray(bv, dtype=np.float32)
    bo = np.asarray(bo, dtype=np.float32)

    variant = _detect_variant(mask)
    nc = _get_nc(variant)
    in_maps = [
        _host_inputs(variant, query, key, value, mask, Wq, bq, Wk, bk, Wv, bv, Wo, bo, c)
        for c in CORES
    ]
    res = run_bass_kernel_spmd(nc, in_maps, core_ids=CORES)

    result = np.empty((S, B, D), dtype=np.float32)
    if variant == "causal":
        bo_eff = (bo + Wo @ bv).astype(np.float32)
        for c in CORES:
            o = res.results[c]["out"].astype(np.float32)
            if bo_eff.any():
                o = o + bo_eff
            result[:, c, :] = o
    else:
        for c in CORES:
            result[:, c, :] = res.results[c]["out"]
    return result
